# revision 3
# baseline (speedup 1.0000x reference)
"""Multi-head self-attention on 8 Trainium2 NeuronCores.

Sharding: batch (2) x head-groups (4 groups of 4 heads) -> 8 cores.
Per core: x[b] @ wq/wk/wv column slices (256 ch), 4 heads of attention,
row-parallel wo -> partial [2048, 1024] output; host sums the 4 group
partials per batch.

Layout notes (per core):
  xT   [1024, 2048]  x[b] transposed (host-side) so d_model is on partitions
  QT/KT [256, 2048]  (x@wq).T computed directly via lhsT=wq chunk
  V    interleaved [2048 t, 4*65]: per head 64 v-cols + a ones column
       (the ones column makes the PV matmul also produce the softmax
        denominator as row 64 of the output)
  scores computed transposed S'[t2, t1]; softmax needs no max-subtraction
  (scores ~ N(0,1)) so P' = exp(S'/8) and the denominator comes from the
  ones column.  attnT [256 c, 2048 t] then feeds wo with natural layouts.
All matmul operands are float32r (full-rate fp32 on the PE array,
~1.5e-4 relative error; fp32 PSUM accumulation).
"""

import sys

sys.path.insert(0, "/opt/trn_rl_repo")

import numpy as np
import concourse.bass as bass
import concourse.mybir as mybir
import concourse.tile as tile
from concourse import bacc
from concourse.bass_utils import run_bass_kernel_spmd

B, T, D = 2, 2048, 1024
NH = 4  # heads per core
HD = 64  # head dim
CH = NH * HD  # 256 channels per core
KD = D // 128  # 8 k-ptiles
CP = CH // 128  # 2 c-ptiles
TP = T // 128  # 16 t-ptiles
TBW = 512  # t1 block width
TB = T // TBW  # 4 t1 blocks
VW = HD + 1  # 65: v columns + ones column
VROW = NH * VW  # 260

F32 = mybir.dt.float32
F32R = mybir.dt.float32r
EXP = mybir.ActivationFunctionType.Exp

_cached_nc = None


def _build():
    nc = bacc.Bacc(None, target_bir_lowering=False)
    xT = nc.dram_tensor("xT", [D, T], F32R, kind="ExternalInput")
    wq = nc.dram_tensor("wq", [D, CH], F32R, kind="ExternalInput")
    wk = nc.dram_tensor("wk", [D, CH], F32R, kind="ExternalInput")
    wv = nc.dram_tensor("wv", [D, CH], F32R, kind="ExternalInput")
    wo = nc.dram_tensor("wo", [CH, D], F32R, kind="ExternalInput")
    ones = nc.dram_tensor("ones", [NH * TP, 128], F32R, kind="ExternalInput")
    y = nc.dram_tensor("y", [T, D], F32, kind="ExternalOutput")

    with tile.TileContext(nc) as tc:
        with tc.tile_pool(name="sb", bufs=1) as sb:
            wot = sb.tile([128, CP * D], F32R)
            qTt = sb.tile([128, CP * T], F32R)
            kTt = sb.tile([128, CP * T], F32R)
            vt = sb.tile([128, TP * VROW], F32R)
            attnT = sb.tile([128, CP * T], F32R)

            # --- projection phase (xT + qkv weights live only here) ---
            proj = tc.tile_pool(name="proj", bufs=1)
            projp = proj.__enter__()
            xTt = projp.tile([128, KD * T], F32R)
            wqt = projp.tile([128, KD * CH], F32R)
            wkt = projp.tile([128, KD * CH], F32R)
            wvt = projp.tile([128, KD * CH], F32R)

            # --- input DMAs ---
            for kd in range(KD):
                nc.sync.dma_start(
                    xTt[:, kd * T : (kd + 1) * T], xT[kd * 128 : (kd + 1) * 128, :]
                )
            for wt_sb, wt_dr in ((wqt, wq), (wkt, wk), (wvt, wv)):
                nc.sync.dma_start(
                    wt_sb.rearrange("p (kd c) -> p kd c", kd=KD),
                    wt_dr.rearrange("(kd p) c -> p kd c", p=128),
                )
            nc.sync.dma_start(
                wot.rearrange("p (kc o) -> p kc o", kc=CP),
                wo.rearrange("(kc p) o -> p kc o", p=128),
            )
            # ones columns of vt: offsets 64 + 65*k, k = 0..NH*TP-1
            nc.sync.dma_start(
                bass.AP(vt.tensor, HD, [[TP * VROW, 128], [VW, NH * TP]]),
                ones.rearrange("k p -> p k"),
            )

            # --- projections ---
            with (
                tc.tile_pool(name="ps_qk", bufs=2, space="PSUM") as ps_qk,
                tc.tile_pool(name="ps_v", bufs=2, space="PSUM") as ps_v,
            ):
                for dst, wsb in ((qTt, wqt), (kTt, wkt)):
                    for cp in range(CP):
                        for tb in range(TB):
                            ps = ps_qk.tile([128, TBW], F32)
                            for kd in range(KD):
                                nc.tensor.matmul(
                                    ps[:],
                                    wsb[:, kd * CH + cp * 128 : kd * CH + cp * 128 + 128],
                                    xTt[:, kd * T + tb * TBW : kd * T + (tb + 1) * TBW],
                                    start=(kd == 0),
                                    stop=(kd == KD - 1),
                                )
                            nc.vector.tensor_copy(
                                dst[:, cp * T + tb * TBW : cp * T + (tb + 1) * TBW],
                                ps[:],
                            )
                for tp in range(TP):
                    ps = ps_v.tile([128, CH], F32)
                    for kd in range(KD):
                        nc.tensor.matmul(
                            ps[:],
                            xTt[:, kd * T + tp * 128 : kd * T + tp * 128 + 128],
                            wvt[:, kd * CH : (kd + 1) * CH],
                            start=(kd == 0),
                            stop=(kd == KD - 1),
                        )
                    # scatter heads into v-interleaved layout [h*65 .. h*65+64)
                    nc.vector.tensor_copy(
                        bass.AP(vt.tensor, tp * VROW, [[TP * VROW, 128], [VW, NH], [1, HD]]),
                        ps[:].rearrange("p (h c) -> p h c", h=NH),
                    )

            proj.__exit__(None, None, None)

            # --- attention ---
            with (
                tc.tile_pool(name="pexp", bufs=2) as pexp,
                tc.tile_pool(name="small", bufs=2) as small,
                tc.tile_pool(name="ps_s", bufs=1, space="PSUM") as ps_s,
                tc.tile_pool(name="ps_o", bufs=1, space="PSUM") as ps_o,
            ):
                for h in range(NH):
                    cp = h // 2
                    po = (h % 2) * 64
                    o_ps = ps_o.tile([VW, T], F32)
                    for i in range(TP):
                        s_ps = ps_s.tile([128, T], F32)
                        for tb in range(TB):
                            nc.tensor.matmul(
                                s_ps[:, tb * TBW : (tb + 1) * TBW],
                                kTt[po : po + 64, cp * T + i * 128 : cp * T + i * 128 + 128],
                                qTt[po : po + 64, cp * T + tb * TBW : cp * T + (tb + 1) * TBW],
                                start=True,
                                stop=True,
                            )
                        pt = pexp.tile([128, T], F32R)
                        nc.scalar.activation(pt[:], s_ps[:], EXP, scale=0.125)
                        for tb in range(TB):
                            nc.tensor.matmul(
                                o_ps[:, tb * TBW : (tb + 1) * TBW],
                                vt[:, i * VROW + VW * h : i * VROW + VW * h + VW],
                                pt[:, tb * TBW : (tb + 1) * TBW],
                                start=(i == 0),
                                stop=(i == TP - 1),
                            )
                    rt = small.tile([1, T], F32)
                    Rt = small.tile([64, T], F32)
                    nc.vector.reciprocal(rt[:], o_ps[64:65, :])
                    nc.gpsimd.partition_broadcast(Rt[:], rt[:])
                    nc.vector.tensor_mul(
                        attnT[po : po + 64, cp * T : (cp + 1) * T], o_ps[0:64, :], Rt[:]
                    )

            # --- output projection ---
            with (
                tc.tile_pool(name="ps_y", bufs=4, space="PSUM") as ps_y,
                tc.tile_pool(name="ystage", bufs=3) as ystage,
            ):
                for tp in range(TP):
                    for ob in range(CP):
                        ps = ps_y.tile([128, TBW], F32)
                        for kc in range(CP):
                            nc.tensor.matmul(
                                ps[:],
                                attnT[:, kc * T + tp * 128 : kc * T + tp * 128 + 128],
                                wot[:, kc * D + ob * TBW : kc * D + (ob + 1) * TBW],
                                start=(kc == 0),
                                stop=(kc == CP - 1),
                            )
                        yt = ystage.tile([128, TBW], F32)
                        nc.vector.tensor_copy(yt[:], ps[:])
                        nc.sync.dma_start(
                            y[tp * 128 : (tp + 1) * 128, ob * TBW : (ob + 1) * TBW],
                            yt[:],
                        )
    nc.compile()
    return nc


def kernel(x, wq, wk, wv, wo, trace=False):
    global _cached_nc
    if _cached_nc is None:
        _cached_nc = _build()
    nc = _cached_nc

    x = np.asarray(x, dtype=np.float32)
    wq = np.asarray(wq, dtype=np.float32)
    wk = np.asarray(wk, dtype=np.float32)
    wv = np.asarray(wv, dtype=np.float32)
    wo = np.asarray(wo, dtype=np.float32)

    ones = np.ones((NH * TP, 128), np.float32)
    in_maps = []
    for c in range(8):
        b, g = c // 4, c % 4
        cs = slice(g * CH, (g + 1) * CH)
        in_maps.append(
            {
                "xT": np.ascontiguousarray(x[b].T),
                "wq": np.ascontiguousarray(wq[:, cs]),
                "wk": np.ascontiguousarray(wk[:, cs]),
                "wv": np.ascontiguousarray(wv[:, cs]),
                "wo": np.ascontiguousarray(wo[cs, :]),
                "ones": ones,
            }
        )

    res = run_bass_kernel_spmd(
        nc, in_maps, core_ids=list(range(8)), trace=trace
    )
    out = np.zeros((B, T, D), np.float32)
    for c in range(8):
        b = c // 4
        out[b] += res.results[c]["y"]
    if trace:
        kernel.last_results = res
    return out


# revision 5
# speedup vs baseline: 1.3191x; 1.3191x over previous
"""Multi-head self-attention on 8 Trainium2 NeuronCores.

Sharding: batch (2) x head-groups (4 groups of 4 heads) -> 8 cores.
Per core: x[b] @ wq/wk/wv column slices (256 ch), 4 heads of attention,
row-parallel wo -> partial [2048, 1024] output; host sums the 4 group
partials per batch.

Layout notes (per core):
  xT   [1024, 2048]  x[b] transposed (host-side) so d_model is on partitions
  QT/KT [256, 2048]  (x@wq).T computed directly via lhsT=wq chunk
  V    interleaved [2048 t, 4*65]: per head 64 v-cols + a ones column
       (the ones column makes the PV matmul also produce the softmax
        denominator as row 64 of the output)
  scores computed transposed S'[t2, t1]; softmax needs no max-subtraction
  (scores ~ N(0,1)) so P' = exp(S'/8) and the denominator comes from the
  ones column.  attnT [256 c, 2048 t] then feeds wo with natural layouts.
All matmul operands are float32r (full-rate fp32 on the PE array,
~1.5e-4 relative error; fp32 PSUM accumulation).

Attention inner structure: per (head, t1-half of 1024): stream over the 16
t2-ptiles with double-buffered S' PSUM tiles ([128,1024], 2 banks each) and
double-buffered PV accumulators ([65,1024], 2 banks each) -> 8 banks total,
keeping the PE busy through the exp (no HAM cold-clocking) and hiding the
per-half normalize chain behind the next half's compute.
"""

import sys

sys.path.insert(0, "/opt/trn_rl_repo")

import numpy as np
import concourse.bass as bass
import concourse.mybir as mybir
import concourse.tile as tile
from concourse import bacc
from concourse.bass_utils import run_bass_kernel_spmd

B, T, D = 2, 2048, 1024
NH = 4  # heads per core
HD = 64  # head dim
CH = NH * HD  # 256 channels per core
KD = D // 128  # 8 k-ptiles
CP = CH // 128  # 2 c-ptiles
TP = T // 128  # 16 t-ptiles
TBW = 512  # matmul free-dim block
TB = T // TBW  # 4
HW_ = 1024  # t1 half width
VW = HD + 1  # 65: v columns + ones column
VROW = NH * VW  # 260

F32 = mybir.dt.float32
F32R = mybir.dt.float32r
EXP = mybir.ActivationFunctionType.Exp

_cached_nc = None


def _build():
    nc = bacc.Bacc(None, target_bir_lowering=False)
    xT = nc.dram_tensor("xT", [D, T], F32R, kind="ExternalInput")
    wq = nc.dram_tensor("wq", [D, CH], F32R, kind="ExternalInput")
    wk = nc.dram_tensor("wk", [D, CH], F32R, kind="ExternalInput")
    wv = nc.dram_tensor("wv", [D, CH], F32R, kind="ExternalInput")
    wo = nc.dram_tensor("wo", [CH, D], F32R, kind="ExternalInput")
    ones = nc.dram_tensor("ones", [NH * TP, 128], F32R, kind="ExternalInput")
    y = nc.dram_tensor("y", [T, D], F32, kind="ExternalOutput")

    with tile.TileContext(nc) as tc:
        with tc.tile_pool(name="sb", bufs=1) as sb:
            wot = sb.tile([128, CP * D], F32R)
            qTt = sb.tile([128, CP * T], F32R)
            kTt = sb.tile([128, CP * T], F32R)
            vt = sb.tile([128, TP * VROW], F32R)
            attnT = sb.tile([128, CP * T], F32R)

            # --- projection phase (xT + qkv weights live only here) ---
            proj = tc.tile_pool(name="proj", bufs=1)
            projp = proj.__enter__()
            wqt = projp.tile([128, KD * CH], F32R)
            wkt = projp.tile([128, KD * CH], F32R)
            wvt = projp.tile([128, KD * CH], F32R)
            xTt = projp.tile([128, KD * T], F32R)

            # --- input DMAs: small weights first so compute starts early ---
            for wt_sb, wt_dr in ((wvt, wv), (wqt, wq), (wkt, wk)):
                nc.sync.dma_start(
                    wt_sb.rearrange("p (kd c) -> p kd c", kd=KD),
                    wt_dr.rearrange("(kd p) c -> p kd c", p=128),
                )
            nc.sync.dma_start(
                wot.rearrange("p (kc o) -> p kc o", kc=CP),
                wo.rearrange("(kc p) o -> p kc o", p=128),
            )
            # ones columns of vt: offsets 64 + 65*k, k = 0..NH*TP-1
            nc.sync.dma_start(
                bass.AP(vt.tensor, HD, [[TP * VROW, 128], [VW, NH * TP]]),
                ones.rearrange("k p -> p k"),
            )
            for kd in range(KD):
                nc.sync.dma_start(
                    xTt[:, kd * T : (kd + 1) * T], xT[kd * 128 : (kd + 1) * 128, :]
                )

            # --- projections (V first: attention consumes it tile-by-tile) ---
            with (
                tc.tile_pool(name="ps_qk", bufs=2, space="PSUM") as ps_qk,
                tc.tile_pool(name="ps_v", bufs=2, space="PSUM") as ps_v,
            ):
                for tp in range(TP):
                    ps = ps_v.tile([128, CH], F32)
                    for kd in range(KD):
                        nc.tensor.matmul(
                            ps[:],
                            xTt[:, kd * T + tp * 128 : kd * T + tp * 128 + 128],
                            wvt[:, kd * CH : (kd + 1) * CH],
                            start=(kd == 0),
                            stop=(kd == KD - 1),
                        )
                    # scatter heads into v-interleaved layout [h*65 .. h*65+64)
                    nc.vector.tensor_copy(
                        bass.AP(vt.tensor, tp * VROW, [[TP * VROW, 128], [VW, NH], [1, HD]]),
                        ps[:].rearrange("p (h c) -> p h c", h=NH),
                    )
                for dst, wsb in ((qTt, wqt), (kTt, wkt)):
                    for cp in range(CP):
                        for tb in range(TB):
                            ps = ps_qk.tile([128, TBW], F32)
                            for kd in range(KD):
                                nc.tensor.matmul(
                                    ps[:],
                                    wsb[:, kd * CH + cp * 128 : kd * CH + cp * 128 + 128],
                                    xTt[:, kd * T + tb * TBW : kd * T + (tb + 1) * TBW],
                                    start=(kd == 0),
                                    stop=(kd == KD - 1),
                                )
                            nc.vector.tensor_copy(
                                dst[:, cp * T + tb * TBW : cp * T + (tb + 1) * TBW],
                                ps[:],
                            )

            proj.__exit__(None, None, None)

            # --- attention ---
            with (
                tc.tile_pool(name="pexp", bufs=3) as pexp,
                tc.tile_pool(name="small", bufs=2) as small,
                tc.tile_pool(name="ps_s", bufs=2, space="PSUM") as ps_s,
                tc.tile_pool(name="ps_o", bufs=2, space="PSUM") as ps_o,
            ):
                for h in range(NH):
                    cp = h // 2
                    po = (h % 2) * 64
                    for th in range(2):  # t1 halves of 1024
                        t1o = cp * T + th * HW_
                        o_ps = ps_o.tile([VW, HW_], F32)
                        for i in range(TP):
                            s_ps = ps_s.tile([128, HW_], F32)
                            for tb in range(2):
                                nc.tensor.matmul(
                                    s_ps[:, tb * TBW : (tb + 1) * TBW],
                                    kTt[po : po + 64, cp * T + i * 128 : cp * T + i * 128 + 128],
                                    qTt[po : po + 64, t1o + tb * TBW : t1o + (tb + 1) * TBW],
                                    start=True,
                                    stop=True,
                                )
                            pt = pexp.tile([128, HW_], F32R)
                            nc.scalar.activation(pt[:], s_ps[:], EXP, scale=0.125)
                            for tb in range(2):
                                nc.tensor.matmul(
                                    o_ps[:, tb * TBW : (tb + 1) * TBW],
                                    vt[:, i * VROW + VW * h : i * VROW + VW * h + VW],
                                    pt[:, tb * TBW : (tb + 1) * TBW],
                                    start=(i == 0),
                                    stop=(i == TP - 1),
                                )
                        rt = small.tile([1, HW_], F32)
                        scr = small.tile([1, HW_], F32)
                        lscr = small.tile([1, HW_], F32)
                        Rt = small.tile([64, HW_], F32)
                        nc.vector.tensor_copy(scr[:], o_ps[64:65, :])
                        nc.vector.reciprocal_approx_accurate(
                            rt[:], scr[:], lscr[:]
                        )
                        nc.gpsimd.partition_broadcast(Rt[:], rt[:])
                        nc.vector.tensor_mul(
                            attnT[po : po + 64, th * HW_ + cp * T : th * HW_ + cp * T + HW_],
                            o_ps[0:64, :],
                            Rt[:],
                        )

            # --- output projection ---
            with (
                tc.tile_pool(name="ps_y", bufs=4, space="PSUM") as ps_y,
                tc.tile_pool(name="ystage", bufs=3) as ystage,
            ):
                for tp in range(TP):
                    for ob in range(CP):
                        ps = ps_y.tile([128, TBW], F32)
                        for kc in range(CP):
                            nc.tensor.matmul(
                                ps[:],
                                attnT[:, kc * T + tp * 128 : kc * T + tp * 128 + 128],
                                wot[:, kc * D + ob * TBW : kc * D + (ob + 1) * TBW],
                                start=(kc == 0),
                                stop=(kc == CP - 1),
                            )
                        yt = ystage.tile([128, TBW], F32)
                        nc.vector.tensor_copy(yt[:], ps[:])
                        nc.sync.dma_start(
                            y[tp * 128 : (tp + 1) * 128, ob * TBW : (ob + 1) * TBW],
                            yt[:],
                        )
    nc.compile()
    return nc


def kernel(x, wq, wk, wv, wo, trace=False):
    global _cached_nc
    if _cached_nc is None:
        _cached_nc = _build()
    nc = _cached_nc

    x = np.asarray(x, dtype=np.float32)
    wq = np.asarray(wq, dtype=np.float32)
    wk = np.asarray(wk, dtype=np.float32)
    wv = np.asarray(wv, dtype=np.float32)
    wo = np.asarray(wo, dtype=np.float32)

    ones = np.ones((NH * TP, 128), np.float32)
    in_maps = []
    for c in range(8):
        b, g = c // 4, c % 4
        cs = slice(g * CH, (g + 1) * CH)
        in_maps.append(
            {
                "xT": np.ascontiguousarray(x[b].T),
                "wq": np.ascontiguousarray(wq[:, cs]),
                "wk": np.ascontiguousarray(wk[:, cs]),
                "wv": np.ascontiguousarray(wv[:, cs]),
                "wo": np.ascontiguousarray(wo[cs, :]),
                "ones": ones,
            }
        )

    res = run_bass_kernel_spmd(
        nc, in_maps, core_ids=list(range(8)), trace=trace
    )
    out = np.zeros((B, T, D), np.float32)
    for c in range(8):
        b = c // 4
        out[b] += res.results[c]["y"]
    if trace:
        kernel.last_results = res
    return out


# revision 7
# speedup vs baseline: 1.3865x; 1.0510x over previous
"""Multi-head self-attention on 8 Trainium2 NeuronCores.

Sharding: batch (2) x head-groups (4 groups of 4 heads) -> 8 cores.
Per core: x[b] @ wq/wk/wv column slices (256 ch), 4 heads of attention,
row-parallel wo -> partial [2048, 1024] output; host sums the 4 group
partials per batch.

Layout notes (per core):
  xT   [1024, 2048]  x[b] transposed (host-side) so d_model is on partitions
  QT/KT [256, 2048]  (x@wq).T computed directly via lhsT=wq chunk
  V    interleaved [2048 t, 4*65]: per head 64 v-cols + a ones column
       (the ones column makes the PV matmul also produce the softmax
        denominator as row 64 of the output)
  scores computed transposed S'[t2, t1]; softmax needs no max-subtraction
  (scores ~ N(0,1)) so P' = exp(S'/8) and the denominator comes from the
  ones column.  attnT [256 c, 2048 t] then feeds wo with natural layouts.
All matmul operands are float32r (full-rate fp32 on the PE array,
~1.5e-4 relative error; fp32 PSUM accumulation).

Attention inner structure: per (head, t1-half of 1024): stream over the 16
t2-ptiles with double-buffered S' PSUM tiles ([128,1024], 2 banks each) and
double-buffered PV accumulators ([65,1024], 2 banks each) -> 8 banks total,
keeping the PE busy through the exp (no HAM cold-clocking) and hiding the
per-half normalize chain behind the next half's compute.
"""

import sys

sys.path.insert(0, "/opt/trn_rl_repo")

import numpy as np
import concourse.bass as bass
import concourse.mybir as mybir
import concourse.tile as tile
from concourse import bacc
from concourse.bass_utils import run_bass_kernel_spmd

B, T, D = 2, 2048, 1024
NH = 4  # heads per core
HD = 64  # head dim
CH = NH * HD  # 256 channels per core
KD = D // 128  # 8 k-ptiles
CP = CH // 128  # 2 c-ptiles
TP = T // 128  # 16 t-ptiles
TBW = 512  # matmul free-dim block
TB = T // TBW  # 4
HW_ = 1024  # t1 half width
VW = HD + 1  # 65: v columns + ones column
VROW = NH * VW  # 260

F32 = mybir.dt.float32
F32R = mybir.dt.float32r
EXP = mybir.ActivationFunctionType.Exp

_cached_nc = None


def _build():
    nc = bacc.Bacc(None, target_bir_lowering=False)
    xT = nc.dram_tensor("xT", [D, T], F32R, kind="ExternalInput")
    wq = nc.dram_tensor("wq", [D, CH], F32R, kind="ExternalInput")
    wk = nc.dram_tensor("wk", [D, CH], F32R, kind="ExternalInput")
    wv = nc.dram_tensor("wv", [D, CH], F32R, kind="ExternalInput")
    wo = nc.dram_tensor("wo", [CH, D], F32R, kind="ExternalInput")
    ones = nc.dram_tensor("ones", [NH * TP, 128], F32R, kind="ExternalInput")
    y = nc.dram_tensor("y", [T, D], F32, kind="ExternalOutput")

    with tile.TileContext(nc) as tc:
        with tc.tile_pool(name="sb", bufs=1) as sb:
            wot = sb.tile([128, CP * D], F32R)
            qTt = sb.tile([128, CP * T], F32R)
            kTt = sb.tile([128, CP * T], F32R)
            vt = sb.tile([128, TP * VROW], F32R)
            attnT = sb.tile([128, CP * T], F32R)

            # --- projection phase (xT + qkv weights live only here) ---
            proj = tc.tile_pool(name="proj", bufs=1)
            projp = proj.__enter__()
            wqt = projp.tile([128, KD * CH], F32R)
            wkt = projp.tile([128, KD * CH], F32R)
            wvt = projp.tile([128, KD * CH], F32R)
            xTt = projp.tile([128, KD * T], F32R)

            # --- input DMAs: small weights first so compute starts early ---
            for wt_sb, wt_dr in ((wvt, wv), (wqt, wq), (wkt, wk)):
                nc.sync.dma_start(
                    wt_sb.rearrange("p (kd c) -> p kd c", kd=KD),
                    wt_dr.rearrange("(kd p) c -> p kd c", p=128),
                )
            nc.sync.dma_start(
                wot.rearrange("p (kc o) -> p kc o", kc=CP),
                wo.rearrange("(kc p) o -> p kc o", p=128),
            )
            # ones columns of vt: offsets 64 + 65*k, k = 0..NH*TP-1
            nc.sync.dma_start(
                bass.AP(vt.tensor, HD, [[TP * VROW, 128], [VW, NH * TP]]),
                ones.rearrange("k p -> p k"),
            )
            for kd in range(KD):
                nc.sync.dma_start(
                    xTt[:, kd * T : (kd + 1) * T], xT[kd * 128 : (kd + 1) * 128, :]
                )

            # --- projections (V first: attention consumes it tile-by-tile) ---
            with (
                tc.tile_pool(name="ps_qk", bufs=2, space="PSUM") as ps_qk,
                tc.tile_pool(name="ps_v", bufs=2, space="PSUM") as ps_v,
            ):
                for tp in range(TP):
                    ps = ps_v.tile([128, CH], F32)
                    for kd in range(KD):
                        nc.tensor.matmul(
                            ps[:],
                            xTt[:, kd * T + tp * 128 : kd * T + tp * 128 + 128],
                            wvt[:, kd * CH : (kd + 1) * CH],
                            start=(kd == 0),
                            stop=(kd == KD - 1),
                        )
                    # scatter heads into v-interleaved layout [h*65 .. h*65+64)
                    nc.vector.tensor_copy(
                        bass.AP(vt.tensor, tp * VROW, [[TP * VROW, 128], [VW, NH], [1, HD]]),
                        ps[:].rearrange("p (h c) -> p h c", h=NH),
                    )
                for dst, wsb in ((qTt, wqt), (kTt, wkt)):
                    for cp in range(CP):
                        for tb in range(TB):
                            ps = ps_qk.tile([128, TBW], F32)
                            for kd in range(KD):
                                nc.tensor.matmul(
                                    ps[:],
                                    wsb[:, kd * CH + cp * 128 : kd * CH + cp * 128 + 128],
                                    xTt[:, kd * T + tb * TBW : kd * T + (tb + 1) * TBW],
                                    start=(kd == 0),
                                    stop=(kd == KD - 1),
                                )
                            nc.vector.tensor_copy(
                                dst[:, cp * T + tb * TBW : cp * T + (tb + 1) * TBW],
                                ps[:],
                            )

            proj.__exit__(None, None, None)

            # --- attention ---
            with (
                tc.tile_pool(name="pexp", bufs=6) as pexp,
                tc.tile_pool(name="small", bufs=2) as small,
                tc.tile_pool(name="ps_s", bufs=2, space="PSUM") as ps_s,
                tc.tile_pool(name="ps_o", bufs=2, space="PSUM") as ps_o,
            ):
                for h in range(NH):
                    cp = h // 2
                    po = (h % 2) * 64
                    for th in range(2):  # t1 halves of 1024
                        t1o = cp * T + th * HW_
                        o_ps = ps_o.tile([VW, HW_], F32)
                        # burst S'/exp vs PV in groups of 4 t2-ptiles so the
                        # PE weight stream doesn't alternate between kT and v
                        # every other matmul (XBUS contention halves the
                        # moving-operand rate for f32r when it does)
                        pts = {}
                        for ii in range(0, TP, 4):
                            for i in range(ii, ii + 4):
                                s_ps = ps_s.tile([128, HW_], F32)
                                for tb in range(2):
                                    nc.tensor.matmul(
                                        s_ps[:, tb * TBW : (tb + 1) * TBW],
                                        kTt[po : po + 64, cp * T + i * 128 : cp * T + i * 128 + 128],
                                        qTt[po : po + 64, t1o + tb * TBW : t1o + (tb + 1) * TBW],
                                        start=True,
                                        stop=True,
                                    )
                                pt = pexp.tile([128, HW_], F32R)
                                nc.scalar.activation(pt[:], s_ps[:], EXP, scale=0.125)
                                pts[i] = pt
                            for i in range(ii, ii + 4):
                                for tb in range(2):
                                    nc.tensor.matmul(
                                        o_ps[:, tb * TBW : (tb + 1) * TBW],
                                        vt[:, i * VROW + VW * h : i * VROW + VW * h + VW],
                                        pts[i][:, tb * TBW : (tb + 1) * TBW],
                                        start=(i == 0),
                                        stop=(i == TP - 1),
                                    )
                        rt = small.tile([1, HW_], F32)
                        scr = small.tile([1, HW_], F32)
                        lscr = small.tile([1, HW_], F32)
                        Rt = small.tile([64, HW_], F32)
                        nc.vector.tensor_copy(scr[:], o_ps[64:65, :])
                        nc.vector.reciprocal_approx_accurate(
                            rt[:], scr[:], lscr[:]
                        )
                        nc.gpsimd.partition_broadcast(Rt[:], rt[:])
                        nc.vector.tensor_mul(
                            attnT[po : po + 64, th * HW_ + cp * T : th * HW_ + cp * T + HW_],
                            o_ps[0:64, :],
                            Rt[:],
                        )

            # --- output projection ---
            with (
                tc.tile_pool(name="ps_y", bufs=4, space="PSUM") as ps_y,
                tc.tile_pool(name="ystage", bufs=3) as ystage,
            ):
                for tp in range(TP):
                    for ob in range(CP):
                        ps = ps_y.tile([128, TBW], F32)
                        for kc in range(CP):
                            nc.tensor.matmul(
                                ps[:],
                                attnT[:, kc * T + tp * 128 : kc * T + tp * 128 + 128],
                                wot[:, kc * D + ob * TBW : kc * D + (ob + 1) * TBW],
                                start=(kc == 0),
                                stop=(kc == CP - 1),
                            )
                        yt = ystage.tile([128, TBW], F32)
                        nc.vector.tensor_copy(yt[:], ps[:])
                        nc.sync.dma_start(
                            y[tp * 128 : (tp + 1) * 128, ob * TBW : (ob + 1) * TBW],
                            yt[:],
                        )
    nc.compile()
    return nc


def kernel(x, wq, wk, wv, wo, trace=False):
    global _cached_nc
    if _cached_nc is None:
        _cached_nc = _build()
    nc = _cached_nc

    x = np.asarray(x, dtype=np.float32)
    wq = np.asarray(wq, dtype=np.float32)
    wk = np.asarray(wk, dtype=np.float32)
    wv = np.asarray(wv, dtype=np.float32)
    wo = np.asarray(wo, dtype=np.float32)

    ones = np.ones((NH * TP, 128), np.float32)
    in_maps = []
    for c in range(8):
        b, g = c // 4, c % 4
        cs = slice(g * CH, (g + 1) * CH)
        in_maps.append(
            {
                "xT": np.ascontiguousarray(x[b].T),
                "wq": np.ascontiguousarray(wq[:, cs]),
                "wk": np.ascontiguousarray(wk[:, cs]),
                "wv": np.ascontiguousarray(wv[:, cs]),
                "wo": np.ascontiguousarray(wo[cs, :]),
                "ones": ones,
            }
        )

    res = run_bass_kernel_spmd(
        nc, in_maps, core_ids=list(range(8)), trace=trace
    )
    out = np.zeros((B, T, D), np.float32)
    for c in range(8):
        b = c // 4
        out[b] += res.results[c]["y"]
    if trace:
        kernel.last_results = res
    return out
